# revision 4
# baseline (speedup 1.0000x reference)
"""Multi-head attention on 8 Trainium2 NeuronCores (Bass/Tile).

Problem: B=4, S=2048, d_model=1024, 16 heads x 64. Full (unsharded) inputs
in, full output out.

Sharding: core c handles batch b=c//2 and head-group g=c%2 (8 of 16 heads);
the output projection is row-sharded and the pair-sum is done on the host
during unsharding (out[b] = part[2b] + part[2b+1] + (bv@Wo + bo), since bv
passes through attention unchanged).

Per-core kernel (all matmuls in fp32r - full-rate fp32 on TRN2):
  x^T via PE transposes; Q^T/K^T in [dq, S] layout, V natural [S, dv].
  Per head: scoresT[k,q] = K_h Q_h^T; P^T = exp(scoresT/8) on ACT straight
  out of PSUM (no max subtraction - |scores| < ~12 is safe in fp32);
  [outT_num; denom] = [V_h*m ; m]^T P^T in one PSUM accumulation group;
  outT = outT_num * (1/denom) with a gpsimd partition-broadcast.
  out_partial = outT^T @ Wo_rows.
Key-side attention mask is folded into V' rows and the denominator column.
"""
import numpy as np

import concourse.bass as bass
import concourse.tile as tile
from concourse import bacc, mybir
from concourse.bass_utils import run_bass_kernel_spmd
from concourse.masks import make_identity

F32 = mybir.dt.float32
F32R = mybir.dt.float32r
AF = mybir.ActivationFunctionType

_S = 2048
_NC_CACHE = {}


def _build(S=_S):
    DM, DQ, H = 1024, 512, 8
    KB, MB = DM // 128, DQ // 128
    NCH, KT, QC = S // 512, S // 128, 512

    nc = bacc.Bacc()
    xb = nc.declare_dram_parameter("xb", [S, DM], F32, isOutput=False)
    wq = nc.declare_dram_parameter("wq", [DM, DQ], F32, isOutput=False)
    wk = nc.declare_dram_parameter("wk", [DM, DQ], F32, isOutput=False)
    wv = nc.declare_dram_parameter("wv", [DM, DQ], F32, isOutput=False)
    wo = nc.declare_dram_parameter("wo", [DQ, DM], F32, isOutput=False)
    bq_pk = nc.declare_dram_parameter("bq_pk", [128, MB], F32, isOutput=False)
    bk_pk = nc.declare_dram_parameter("bk_pk", [128, MB], F32, isOutput=False)
    mv_pk = nc.declare_dram_parameter("mv_pk", [128, KT], F32, isOutput=False)
    out = nc.declare_dram_parameter("out", [S, DM], F32, isOutput=True)

    with tile.TileContext(nc) as tc:
        with tc.tile_pool(name="persist", bufs=1) as pp:
            ident = pp.tile([128, 128], F32, tag="ident")
            make_identity(nc, ident)
            bq_sb = pp.tile([128, MB], F32, tag="bq")
            bk_sb = pp.tile([128, MB], F32, tag="bk")
            mv_sb = pp.tile([128, KT], F32, tag="mv")
            nc.sync.dma_start(bq_sb, bq_pk[:])
            nc.sync.dma_start(bk_sb, bk_pk[:])
            nc.sync.dma_start(mv_sb, mv_pk[:])

            # V' [128, kt, h, 66]: per head 64 v-dims + mask col (65th) + pad
            vp = pp.tile([128, KT, H, 65], F32R, tag="vp")
            # attention numerator/denominator output, transposed layout
            ot = pp.tile([128, MB, S], F32R, tag="ot")

            with tc.tile_pool(name="qk", bufs=1) as qkp:
                qt = qkp.tile([128, MB, S], F32R, tag="qt")
                kt_t = qkp.tile([128, MB, S], F32R, tag="kt")

                # ---------------- Phase 1: x^T, QKV projections ----------
                with (
                    tc.tile_pool(name="ph1", bufs=2) as p1,
                    tc.tile_pool(name="wpool", bufs=1) as wp,
                    tc.tile_pool(name="ph1ps", bufs=2, space="PSUM") as tps,
                    tc.tile_pool(name="qkvps", bufs=3, space="PSUM") as qps,
                ):
                    w_r = {}
                    for name, w_h in (("q", wq), ("k", wk), ("v", wv)):
                        w_r[name] = wp.tile([128, KB, DQ], F32R,
                                            tag=f"w{name}", name=f"w{name}")
                        nc.sync.dma_start(
                            w_r[name],
                            w_h.ap().bitcast(F32R).rearrange(
                                "(kb p) n -> p kb n", p=128))

                    QC1 = 256   # phase-1 S-chunk (SBUF pressure)
                    for n in range(S // QC1):
                        xt_c = p1.tile([128, KB, QC1], F32R, tag="xt")
                        for st in range(QC1 // 128):
                            x_nat = p1.tile([128, DM], F32, tag="xnat")
                            row0 = n * QC1 + st * 128
                            nc.sync.dma_start(x_nat, xb.ap()[row0:row0 + 128, :])
                            for dj in range(KB):
                                tp = tps.tile([128, 128], F32, tag="tp")
                                nc.tensor.transpose(
                                    tp, x_nat[:, dj * 128:(dj + 1) * 128], ident)
                                nc.vector.tensor_copy(
                                    out=xt_c[:, dj, st * 128:(st + 1) * 128], in_=tp)
                        for name, dst, bias in (("q", qt, bq_sb), ("k", kt_t, bk_sb)):
                            for m in range(MB):
                                pq = qps.tile([128, QC1], F32, tag="pqkv")
                                for dj in range(KB):
                                    nc.tensor.matmul(
                                        pq,
                                        w_r[name][:, dj, m * 128:(m + 1) * 128],
                                        xt_c[:, dj, :],
                                        start=(dj == 0), stop=(dj == KB - 1))
                                nc.vector.tensor_scalar_add(
                                    out=dst[:, m, n * QC1:(n + 1) * QC1],
                                    in0=pq, scalar1=bias[:, m:m + 1])
                        for st in range(QC1 // 128):
                            ktile = n * (QC1 // 128) + st
                            pv = qps.tile([128, DQ], F32, tag="pqkv")
                            for dj in range(KB):
                                nc.tensor.matmul(
                                    pv,
                                    xt_c[:, dj, st * 128:(st + 1) * 128],
                                    w_r["v"][:, dj, :],
                                    start=(dj == 0), stop=(dj == KB - 1))
                            nc.vector.tensor_scalar_mul(
                                out=vp[:, ktile, :, 0:64],
                                in0=pv.rearrange("p (h d) -> p h d", h=H),
                                scalar1=mv_sb[:, ktile:ktile + 1])
                            nc.vector.tensor_copy(
                                out=vp[:, ktile, :, 64:65],
                                in_=mv_sb[:, ktile:ktile + 1, None].to_broadcast(
                                    (128, H, 1)))

                # ---------------- Phase 2: attention ---------------------
                with (
                    tc.tile_pool(name="attn", bufs=2) as ap,
                    tc.tile_pool(name="scps", bufs=3, space="PSUM") as sps,
                    tc.tile_pool(name="pvps", bufs=2, space="PSUM") as ops,
                ):
                    LAG = 2
                    for h in range(H):
                        hb, po = h // 2, 64 * (h % 2)
                        for q in range(NCH):
                            qs = slice(q * QC, (q + 1) * QC)
                            po_t = ops.tile([128, QC], F32, tag="po")
                            pts = {}

                            def pv_step(ktile, po_t=po_t, h=h, pts=pts):
                                nc.tensor.matmul(
                                    po_t[0:65],
                                    vp[:, ktile, h, 0:65],
                                    pts.pop(ktile),
                                    start=(ktile == 0), stop=(ktile == KT - 1))

                            for ktile in range(KT):
                                ps_s = sps.tile([128, QC], F32, tag="ps")
                                nc.tensor.matmul(
                                    ps_s,
                                    kt_t[po:po + 64, hb,
                                         ktile * 128:(ktile + 1) * 128],
                                    qt[po:po + 64, hb, qs],
                                    start=True, stop=True)
                                ptk = ap.tile([128, QC], F32R, tag="pt",
                                              bufs=LAG + 2)
                                nc.scalar.activation(
                                    ptk, ps_s, AF.Exp, scale=0.125)
                                pts[ktile] = ptk
                                if ktile >= LAG:
                                    pv_step(ktile - LAG)
                            for ktile in range(KT - LAG, KT):
                                pv_step(ktile)
                            den = ap.tile([128, QC], F32, tag="den")
                            rec = ap.tile([128, QC], F32, tag="rec")
                            rep = ap.tile([64, QC], F32, tag="rep")
                            nc.vector.tensor_copy(out=den[64:65],
                                                  in_=po_t[64:65])
                            nc.sync.dma_start(den[0:1], den[64:65])
                            nc.vector.reciprocal(rec[0:1], den[0:1])
                            nc.gpsimd.partition_broadcast(
                                rep, rec[0:1], channels=64)
                            if po == 0:
                                nc.vector.tensor_mul(
                                    out=ot[0:64, hb, qs],
                                    in0=po_t[0:64], in1=rep)
                            else:
                                shf = ap.tile([64, QC], F32R, tag="shf")
                                nc.vector.tensor_mul(
                                    out=shf, in0=po_t[0:64], in1=rep)
                                nc.sync.dma_start(ot[64:128, hb, qs], shf)

            # ---------------- Phase 3: output projection -----------------
            with (
                tc.tile_pool(name="proj", bufs=2) as prp,
                tc.tile_pool(name="wop", bufs=1) as wop,
                tc.tile_pool(name="prps", bufs=3, space="PSUM") as fps,
            ):
                wo_r = wop.tile([128, MB, DM], F32R, tag="wo")
                nc.sync.dma_start(
                    wo_r,
                    wo.ap().bitcast(F32R).rearrange("(m p) n -> p m n", p=128))
                for qt_i in range(S // 128):
                    for ncb in range(2):
                        ns = slice(ncb * 512, (ncb + 1) * 512)
                        pf = fps.tile([128, 512], F32, tag="pf")
                        for m in range(MB):
                            nc.tensor.matmul(
                                pf,
                                ot[:, m, qt_i * 128:(qt_i + 1) * 128],
                                wo_r[:, m, ns],
                                start=(m == 0), stop=(m == MB - 1))
                        o_st = prp.tile([128, 512], F32, tag="ost")
                        nc.vector.tensor_copy(out=o_st, in_=pf)
                        nc.sync.dma_start(
                            out.ap()[qt_i * 128:(qt_i + 1) * 128, ns], o_st)

    nc.compile()
    return nc


def get_nc(S=_S):
    if S not in _NC_CACHE:
        _NC_CACHE[S] = _build(S)
    return _NC_CACHE[S]


def shard_inputs(inputs, S=_S):
    x = np.asarray(inputs["x"], dtype=np.float32)
    mask = np.asarray(inputs["attention_mask"])
    Wq, Wk, Wv, Wo = (np.asarray(inputs[k], dtype=np.float32)
                      for k in ("Wq", "Wk", "Wv", "Wo"))
    bq, bk, bv, bo = (np.asarray(inputs[k], dtype=np.float32)
                      for k in ("bq", "bk", "bv", "bo"))
    in_maps = []
    for c in range(8):
        b, g = c // 2, c % 2
        cols = slice(g * 512, (g + 1) * 512)
        in_maps.append({
            "xb": np.ascontiguousarray(x[b, :S]),
            "wq": np.ascontiguousarray(Wq[:, cols]),
            "wk": np.ascontiguousarray(Wk[:, cols]),
            "wv": np.ascontiguousarray(Wv[:, cols]),
            "wo": np.ascontiguousarray(Wo[cols, :]),
            "bq_pk": np.ascontiguousarray(bq[cols].reshape(4, 128).T),
            "bk_pk": np.ascontiguousarray(bk[cols].reshape(4, 128).T),
            "mv_pk": np.ascontiguousarray(
                mask[b, :S].astype(np.float32).reshape(S // 128, 128).T),
        })
    host_bias = bv @ Wo + bo   # bv passes through attention unchanged
    return in_maps, host_bias


def unshard_outputs(results, host_bias, S=_S):
    out = np.empty((4, S, 1024), dtype=np.float32)
    for b in range(4):
        out[b] = results[2 * b]["out"] + results[2 * b + 1]["out"] + host_bias
    return out


def kernel(**inputs):
    nc = get_nc()
    in_maps, host_bias = shard_inputs(inputs)
    res = run_bass_kernel_spmd(nc, in_maps, core_ids=list(range(8)))
    return unshard_outputs(res.results, host_bias)
